# revision 12
# baseline (speedup 1.0000x reference)
"""DGCNN-simple Trainium2 kernel (v2: chunked quantized-encode KNN).

Strategy (8 NeuronCores, B=4 samples):
  core c -> sample b = c//2, query half h = c%2 (2048 queries each).
  Per EdgeConv, conv+BN+LReLU+max over K neighbors folded via LReLU
  monotonicity:
     x_out(o,n) = LReLU( max_k P(o, j_k(n)) + Q(o,n) )
  with P = (s*A) @ x (neighbor part, DRAM table) and Q = (s*(B-A)) @ x + t.

  KNN (the hot loop) uses an approximate-but-validated scheme:
  the pd matmul computes  S*pd + 1.5*2^32  (scale S and the big constant
  folded into the contraction as extra rows; the big constant is the LAST
  contraction row).  fp32 rounding in the [2^32, 2^33) binade quantizes the
  PSUM value to multiples of 512 "for free".  One scalar_tensor_tensor per
  512-chunk then computes  d = (c - (1.5*2^32 - 2^23)) + (j mod 512)
  which embeds the within-chunk column index into the low 9 bits exactly.
  A single max8 per 512-chunk (8 DVE passes of 512 = 1 full-width pass
  instead of 8) gives 64 candidates; a narrow 3-round top-20 merge over
  the 64 candidates recovers winners; local index = d mod 512 and chunk =
  merge_position//8 reconstruct the global neighbor index exactly.
  Validated vs the exact reference in numpy: rel err 1.25e-3 (gate 2e-2);
  per-row kNN set mismatch 0.22/0.24 of 20.

  Neighbor gather: ONE dma_gather (2560 indices) per query tile instead of
  20 indirect DMAs; fold max via a single strided tensor_reduce.
  Wide matmuls run as float32r (1 cycle/row vs 4 for fp32).
"""

import numpy as np
import concourse.bass as bass
import concourse.bacc as bacc
import concourse.mybir as mybir
import concourse.tile as tile
from concourse.bass_utils import run_bass_kernel_spmd
from concourse.masks import make_identity

N = 4096
K = 20
B = 4
EPS = 1e-5
SLOPE = 0.2
NCORES = 8
HALF = N // 2
P128 = 128
NT = HALF // P128  # query tiles per core
NCH = 8            # knn chunks per row
CW = N // NCH      # chunk width (512)
F32 = mybir.dt.float32
F32R = mybir.dt.float32r
I16 = mybir.dt.int16
U32 = mybir.dt.uint32

# knn quantization scales (validated in numpy): s1=1024, s2=64
S1 = 512.0 * 1024.0
S2 = 512.0 * 64.0
BIG = 1.5 * 2.0**32          # 6442450944, exactly representable
KSUB = BIG - 2.0**23         # subtracted on the encode op
NEG = -1e30

# engine split tuning: how many of the NCH chunk-encodes run on DVE (rest Pool)
ENC_DVE = 0

_BUILT = {}


def _lrelu(nc, out_ap, in_ap):
    nc.vector.scalar_tensor_tensor(out=out_ap, in0=in_ap, scalar=SLOPE, in1=in_ap,
                                   op0=mybir.AluOpType.mult, op1=mybir.AluOpType.max)


def _r(ap):
    return ap.bitcast(F32R)


def _edgeconv_phase(nc, tc, pools, *, lhs, rhs, cdim, ptab_dram, bq_rhs, bq_cdim,
                    lhsq, out_cb, iotaf, dram, ksubv):
    """One EdgeConv over NT query tiles."""
    sb, ps, pd_ps, stg, sbbig = pools
    for t in range(NT):
        q0 = t * P128
        # chunk top-8 (encoded d values carry quantized pd + local idx).
        # gpsimd can't read PSUM: Act moves psum->SBUF with -KSUB folded in,
        # Pool adds the iota in place, DVE max8 scans each 512 chunk.
        V = sb.tile([P128, NCH * 8], F32, tag="V")
        for c4 in range(NCH // 2):
            c0 = c4 * 2 * CW
            pq = pd_ps.tile([P128, 2 * CW], F32, tag="pdq")
            for j in range(2):
                nc.tensor.matmul(pq[:, j * CW:(j + 1) * CW],
                                 _r(lhs[0:cdim, q0:q0 + P128]),
                                 _r(rhs[0:cdim, c0 + j * CW:c0 + (j + 1) * CW]),
                                 start=True, stop=True)
            dch = sbbig.tile([P128, 2 * CW], F32, tag="dch")
            nc.scalar.activation(dch[:], pq[:],
                                 mybir.ActivationFunctionType.Identity,
                                 bias=ksubv[:, 0:1])
            nc.gpsimd.tensor_tensor(out=dch[:], in0=dch[:], in1=iotaf[:],
                                    op=mybir.AluOpType.add)
            for j in range(2):
                nc.vector.max(out=V[:, (2 * c4 + j) * 8:(2 * c4 + j) * 8 + 8],
                              in_=dch[:, j * CW:(j + 1) * CW])
        # narrow 3-round top-24 merge with positions
        sel = sb.tile([P128, 24], F32, tag="sel")
        pos = sb.tile([P128, 24], U32, tag="pos")
        for r in range(3):
            nc.vector.max(out=sel[:, 8 * r:8 * r + 8], in_=V[:])
            nc.vector.max_index(out=pos[:, 8 * r:8 * r + 8],
                                in_max=sel[:, 8 * r:8 * r + 8], in_values=V[:])
            if r < 2:
                nc.vector.match_replace(out=V[:], in_to_replace=sel[:, 8 * r:8 * r + 8],
                                        in_values=V[:], imm_value=NEG)
        # reconstruct global indices: 512*(pos//8) + (sel mod 512)
        localf = sb.tile([P128, K], F32, tag="localf")
        nc.vector.tensor_scalar(out=localf[:], in0=sel[:, 0:K], scalar1=512.0,
                                scalar2=None, op0=mybir.AluOpType.mod)
        posf = sb.tile([P128, K], F32, tag="posf")
        nc.vector.tensor_scalar(out=posf[:], in0=pos[:, 0:K].bitcast(mybir.dt.int32),
                                scalar1=0, scalar2=None, op0=mybir.AluOpType.add)
        m8f = sb.tile([P128, K], F32, tag="m8f")
        nc.vector.tensor_scalar(out=m8f[:], in0=posf[:], scalar1=8.0,
                                scalar2=None, op0=mybir.AluOpType.mod)
        t1f = sb.tile([P128, K], F32, tag="t1f")
        nc.vector.scalar_tensor_tensor(out=t1f[:], in0=m8f[:], scalar=-64.0,
                                       in1=localf[:], op0=mybir.AluOpType.mult,
                                       op1=mybir.AluOpType.add)
        gf = sb.tile([P128, K + 12], F32, tag="gf")
        nc.vector.scalar_tensor_tensor(out=gf[:, 0:K], in0=posf[:], scalar=64.0,
                                       in1=t1f[:], op0=mybir.AluOpType.mult,
                                       op1=mybir.AluOpType.add)
        gi16 = sb.tile([P128, K], I16, tag="gi16")
        nc.vector.tensor_scalar(out=gi16[:], in0=gf[:, 0:K], scalar1=0.0,
                                scalar2=float(N - 1), op0=mybir.AluOpType.max,
                                op1=mybir.AluOpType.min)
        # wrapped-idx layout via DRAM hop:
        # 1) shuffled write: idxd[(q%16)*160 + k*8 + q//16] = gi16[q][k]
        # 2) read back into (128,160): stride-0 leading dim replicates x8
        idxd = dram.tile([1, 16 * 8 * K], I16, tag="idxd")
        nc.sync.dma_start(
            idxd[:].rearrange("one (p k s0) -> one s0 p k", p=16, k=K, s0=8),
            gi16[:])
        idx_sb = sbbig.tile([P128, 8 * K], I16, tag="idxsb")
        nc.sync.dma_start(idx_sb[:],
                          idxd[:].broadcast_to([8, 16 * 8 * K]))
        # one gather for all 20 neighbors x 128 queries
        gath = sbbig.tile([P128, K * 64], F32, tag="gath")
        nc.gpsimd.dma_gather(out_ap=gath[:].rearrange("p (k o) -> p k o", k=K),
                             in_ap=ptab_dram[:], idxs_ap=idx_sb[:],
                             num_idxs=K * P128, num_idxs_reg=K * P128,
                             elem_size=64)
        # fold max over k (log-tree on Pool; DVE is the hot engine)
        def fmax(d0, s0, n):
            nc.gpsimd.tensor_tensor(out=gath[:, d0:d0 + n], in0=gath[:, d0:d0 + n],
                                    in1=gath[:, s0:s0 + n], op=mybir.AluOpType.max)
        fmax(0, 640, 640)    # k0-9  |= k10-19
        fmax(0, 320, 320)    # b0-4  |= b5-9
        fmax(0, 256, 64)     # b0    |= b4
        fmax(0, 128, 128)    # b0,b1 |= b2,b3
        fmax(0, 64, 64)      # b0    |= b1
        m = gath[:, 0:64]
        qp = ps.tile([P128, 64], F32, tag="ps")
        nc.tensor.matmul(qp[:], _r(lhsq[0:bq_cdim, q0:q0 + P128]),
                         _r(bq_rhs[0:bq_cdim, :]), start=True, stop=True)
        nc.vector.tensor_add(out=m[:], in0=m[:], in1=qp[:])
        xt = sb.tile([P128, 65], F32, tag="xoq", name="xoq")
        _lrelu(nc, xt[:, 0:64], m[:])
        out_cb(t, xt)


def _ptable(nc, pools, a_rhs, cdim, src, identity, ptab_dram, ptab_stage):
    """P = A^T-weights @ src -> transpose -> DRAM table (N, 64)."""
    sb, ps, pd_ps, stg, sbbig = pools
    p_sb = stg.tile([64, N], F32, tag="p_sb")
    for j in range(N // 512):
        pp = ps.tile([64, 512], F32, tag="ps")
        nc.tensor.matmul(pp[:], _r(a_rhs[0:cdim, :]), _r(src[0:cdim, j * 512:(j + 1) * 512]),
                         start=True, stop=True)
        nc.scalar.copy(p_sb[:, j * 512:(j + 1) * 512], pp[:])
    for j1 in range(N // P128):
        tp = ps.tile([P128, P128], F32, tag="ps")
        nc.tensor.transpose(tp[:, 0:64], p_sb[:, j1 * P128:(j1 + 1) * P128],
                            identity[0:64, 0:64])
        nc.scalar.copy(ptab_stage[:, j1 * 64:(j1 + 1) * 64], tp[:, 0:64])
    nc.sync.dma_start(
        ptab_dram[:].rearrange("(j1 j0) o -> j0 j1 o", j0=P128),
        ptab_stage[:].rearrange("p (j1 o) -> p j1 o", o=64))


def build():
    nc = bacc.Bacc(None, target_bir_lowering=False)
    dt = F32
    # ---- per-core inputs ----
    xq = nc.dram_tensor("xq", [2, HALF], dt, kind="ExternalInput")
    xc = nc.dram_tensor("xc", [2, N], dt, kind="ExternalInput")
    a1t = nc.dram_tensor("a1t", [2, 64], dt, kind="ExternalInput")
    b1t = nc.dram_tensor("b1t", [5, 64], dt, kind="ExternalInput")
    a2t = nc.dram_tensor("a2t", [64, 64], dt, kind="ExternalInput")
    b2t = nc.dram_tensor("b2t", [66, 64], dt, kind="ExternalInput")
    w5t = nc.dram_tensor("w5t", [128, 128], dt, kind="ExternalInput")
    t5 = nc.dram_tensor("t5", [128, 1], dt, kind="ExternalInput")
    w6at = nc.dram_tensor("w6at", [128, 256], dt, kind="ExternalInput")
    w6bt = nc.dram_tensor("w6bt", [128, 256], dt, kind="ExternalInput")
    t6 = nc.dram_tensor("t6", [128, 2], dt, kind="ExternalInput")
    w9t = nc.dram_tensor("w9t", [128, 2], dt, kind="ExternalInput")
    out = nc.dram_tensor("out", [1, HALF], dt, kind="ExternalOutput")

    with tile.TileContext(nc) as tc:
        with tc.tile_pool(name="sb", bufs=3) as sb, \
             tc.tile_pool(name="sbbig", bufs=3) as sbbig, \
             tc.tile_pool(name="lr", bufs=1) as lrpool, \
             tc.tile_pool(name="stg", bufs=1) as stg, \
             tc.tile_pool(name="sbp", bufs=1) as sbp, \
             tc.tile_pool(name="ps", bufs=2, space="PSUM") as ps, \
             tc.tile_pool(name="pdps", bufs=3, space="PSUM") as pd_ps, \
             tc.tile_pool(name="dram", bufs=3, space="DRAM") as dram:
            pools = (sb, ps, pd_ps, stg, sbbig)
            ident = sbp.tile([P128, P128], F32, tag="ident")
            make_identity(nc, ident[:])

            ksubv = sbp.tile([P128, 1], F32, tag="ksubv")
            nc.vector.memset(ksubv[:], -KSUB)
            # iota constant: (j mod 512) over a 1024 window, fp32 exact
            iotaf = sbp.tile([P128, 2 * CW], F32, tag="iotaf")
            nc.gpsimd.iota(iotaf[:].rearrange("p (a b) -> p a b", b=CW),
                           pattern=[[0, 2], [1, CW]], base=0,
                           channel_multiplier=0,
                           allow_small_or_imprecise_dtypes=True)

            # ---------- load inputs / small weights ----------
            def load(pool, dr, shape, tag, rr=False):
                t = pool.tile(list(shape), F32, tag=tag)
                if rr:
                    nc.sync.dma_start(_r(t[:]), _r(dr[:]))
                else:
                    nc.sync.dma_start(t[:], dr[:])
                return t

            a1s = load(sbp, a1t, (2, 64), "a1", rr=True)
            b1s = load(sbp, b1t, (5, 64), "b1", rr=True)
            a2s = load(sbp, a2t, (64, 64), "a2", rr=True)
            b2s = load(sbp, b2t, (66, 64), "b2", rr=True)
            w5s = load(sbp, w5t, (128, 128), "w5", rr=True)
            t5s = load(sbp, t5, (128, 1), "t5")
            w6as = load(sbp, w6at, (128, 256), "w6a")
            w6bs = load(sbp, w6bt, (128, 256), "w6b", rr=True)
            t6s = load(sbp, t6, (128, 2), "t6")
            w9s = load(sbp, w9t, (128, 2), "w9")

            # ---------- phase A prep ----------
            # lhsA (5, HALF): [2*S1*xq(2); -S1*xxq; S1; BIG]
            # rhsA (5, N):    [xc(2); ones; -xxc; ones]
            lhsA = lrpool.tile([67, HALF], F32, tag="lhsbuf")
            rhsA = lrpool.tile([67, N], F32, tag="rhsbuf")
            nc.sync.dma_start(_r(lhsA[0:2, :]), _r(xq[:]))
            nc.sync.dma_start(_r(rhsA[0:2, :]), _r(xc[:]))
            # const rows: engine ops can't start at odd partitions -> build on
            # partition 0 and DMA into place
            ones_row = sbp.tile([1, N], F32, tag="ones_row")
            nc.vector.memset(ones_row[:], 1.0)
            crow = stg.tile([1, HALF], F32, tag="crow", name="crowS1")
            nc.vector.memset(crow[:], S1)
            nc.sync.dma_start(_r(rhsA[2:3, :]), _r(ones_row[:]))
            nc.sync.dma_start(_r(rhsA[4:5, :]), _r(ones_row[:]))
            nc.sync.dma_start(_r(lhsA[3:4, :]), _r(crow[:]))
            crow2 = stg.tile([1, HALF], F32, tag="crow", name="crowBIG")
            nc.vector.memset(crow2[:], BIG)
            nc.sync.dma_start(_r(lhsA[4:5, :]), _r(crow2[:]))
            # squared norms via ones-matmul on PE, chunked
            ones2 = sbp.tile([2, 1], F32, tag="ones2")
            nc.vector.memset(ones2[:], 1.0)
            for src_ap, dst_row, ncols, scl, nmname in (
                    (lhsA, lhsA[2:3, :], HALF, -S1, "nrmq"),
                    (rhsA, rhsA[3:4, :], N, -1.0, "nrmc")):
                nrow = stg.tile([1, ncols], F32, tag="nrow", name=nmname)
                for j in range(ncols // 512):
                    sl = slice(j * 512, (j + 1) * 512)
                    sqt = stg.tile([2, 512], F32, tag="sqt", name="sqt")
                    nc.vector.tensor_mul(sqt[:], src_ap[0:2, sl], src_ap[0:2, sl])
                    pp = ps.tile([1, 512], F32, tag="ps", name="pp")
                    nc.tensor.matmul(pp[:], ones2[:], sqt[:], start=True, stop=True)
                    nc.scalar.activation(nrow[:, sl], pp[:],
                                         mybir.ActivationFunctionType.Copy, scale=scl)
                nc.sync.dma_start(_r(dst_row[:]), _r(nrow[:]))
            # query rows: 2*S1*xq
            nc.vector.tensor_scalar_mul(_r(lhsA[0:2, :]), lhsA[0:2, :], 2.0 * S1)

            # ---------- P1 table ----------
            p1d = dram.tile([N, 64], F32, tag="p1d")
            pstage = stg.tile([P128, (N // P128) * 64], F32, tag="pstage")
            _ptable(nc, pools, a1s, 2, rhsA, ident, p1d, pstage)

            # ---------- EdgeConv 1 ----------
            x1aug = sbp.tile([66, HALF], F32, tag="x1aug")

            def cb1(t, xt):
                sq = sb.tile([P128, 64], F32, tag="sq64", name="sq64")
                nc.vector.tensor_mul(sq[:], xt[:, 0:64], xt[:, 0:64])
                nc.vector.tensor_reduce(out=xt[:, 64:65], in_=sq[:],
                                        axis=mybir.AxisListType.X,
                                        op=mybir.AluOpType.add, negate=True)
                tp = ps.tile([P128, P128], F32, tag="ps", name="tp1")
                nc.tensor.transpose(tp[0:65, :], xt[:, 0:65], ident[:])
                nc.scalar.copy(_r(x1aug[0:65, t * P128:(t + 1) * P128]), tp[0:65, :])

            _edgeconv_phase(nc, tc, pools, lhs=lhsA, rhs=rhsA, cdim=5,
                            ptab_dram=p1d, bq_rhs=b1s, bq_cdim=5, lhsq=lhsA,
                            out_cb=cb1, iotaf=iotaf, dram=dram, ksubv=ksubv)
            nc.sync.dma_start(_r(x1aug[65:66, :]), _r(ones_row[:, 0:HALF]))

            # ---------- exchange halves (AllGather over core pairs) ----------
            ccin = dram.tile([66, HALF], F32, tag="ccin")
            ccout = dram.tile([132, HALF], F32, tag="ccout")
            nc.sync.dma_start(ccin[:], x1aug[0:66, :])
            nc.gpsimd.collective_compute(
                "AllGather", mybir.AluOpType.bypass,
                replica_groups=[[0, 1], [2, 3], [4, 5], [6, 7]],
                ins=[ccin.opt()], outs=[ccout.opt()])

            # ---------- phase B prep ----------
            # rhsB (67, N): [x1_full(64); ones; -xx1_full; ones]
            rhsB = lrpool.tile([67, N], F32, tag="rhsbuf")
            nc.sync.dma_start(_r(rhsB[0:64, 0:HALF]), _r(ccout[0:64, :]))
            nc.sync.dma_start(_r(rhsB[0:64, HALF:N]), _r(ccout[66:130, :]))
            nc.sync.dma_start(_r(rhsB[65:66, 0:HALF]), _r(ccout[64:65, :]))
            nc.sync.dma_start(_r(rhsB[65:66, HALF:N]), _r(ccout[130:131, :]))
            nc.vector.memset(_r(rhsB[64:65, :]), 1.0)
            nc.sync.dma_start(_r(rhsB[66:67, :]), _r(ones_row[:]))
            # lhsB (67, HALF): [2*S2*x1_own(64); -S2*xx2_own; S2; BIG]
            lhsB = lrpool.tile([67, HALF], F32, tag="lhsbuf")
            nc.vector.tensor_scalar_mul(_r(lhsB[0:64, :]), x1aug[0:64, :], 2.0 * S2)
            nc.vector.tensor_scalar_mul(_r(lhsB[64:65, :]), x1aug[64:65, :], S2)
            crow3 = stg.tile([1, HALF], F32, tag="crow", name="crowS2")
            nc.vector.memset(crow3[:], S2)
            nc.sync.dma_start(_r(lhsB[65:66, :]), _r(crow3[:]))
            crow4 = stg.tile([1, HALF], F32, tag="crow", name="crowBIG2")
            nc.vector.memset(crow4[:], BIG)
            nc.sync.dma_start(_r(lhsB[66:67, :]), _r(crow4[:]))

            # ---------- P2 table ----------
            p2d = dram.tile([N, 64], F32, tag="p2d")
            _ptable(nc, pools, a2s, 64, rhsB, ident, p2d, pstage)

            # ---------- EdgeConv 2 ----------
            xcown = sbp.tile([P128, HALF], F32, tag="xcown")
            nc.scalar.copy(_r(xcown[0:64, :]), x1aug[0:64, :])

            def cb2(t, xt):
                tp = ps.tile([P128, P128], F32, tag="ps", name="tp2")
                nc.tensor.transpose(tp[0:64, :], xt[:, 0:64], ident[:])
                x2s = sb.tile([64, P128], F32, tag="x2s", name="x2s")
                nc.scalar.copy(x2s[:], tp[0:64, :])
                nc.sync.dma_start(_r(xcown[64:128, t * P128:(t + 1) * P128]), _r(x2s[:]))

            _edgeconv_phase(nc, tc, pools, lhs=lhsB, rhs=rhsB, cdim=67,
                            ptab_dram=p2d, bq_rhs=b2s, bq_cdim=66, lhsq=x1aug,
                            out_cb=cb2, iotaf=iotaf, dram=dram, ksubv=ksubv)

            # ---------- tail ----------
            # conv5 + global max (partial over own half)
            h5 = lrpool.tile([P128, HALF], F32, tag="hbuf")
            for j in range(HALF // 512):
                pp = ps.tile([P128, 512], F32, tag="ps")
                nc.tensor.matmul(pp[:], _r(w5s[:]), _r(xcown[:, j * 512:(j + 1) * 512]),
                                 start=True, stop=True)
                sl5 = h5[:, j * 512:(j + 1) * 512]
                nc.scalar.activation(sl5, pp[:], mybir.ActivationFunctionType.Identity,
                                     bias=t5s[:, 0:1])
                _lrelu(nc, sl5, sl5)
            gpart = sb.tile([P128, 1], F32, tag="gpart")
            nc.vector.tensor_reduce(out=gpart[:], in_=h5[:],
                                    axis=mybir.AxisListType.X, op=mybir.AluOpType.max)
            gin = dram.tile([P128, 1], F32, tag="gin")
            gout = dram.tile([256, 1], F32, tag="gout")
            nc.sync.dma_start(gin[:], gpart[:])
            nc.gpsimd.collective_compute(
                "AllGather", mybir.AluOpType.bypass,
                replica_groups=[[0, 1], [2, 3], [4, 5], [6, 7]],
                ins=[gin.opt()], outs=[gout.opt()])
            gboth = sb.tile([P128, 2], F32, tag="gboth")
            nc.sync.dma_start(gboth[:, 0:1], gout[0:P128, :])
            nc.sync.dma_start(gboth[:, 1:2], gout[P128:256, :])
            gmax = sb.tile([P128, 1], F32, tag="gmax")
            nc.vector.tensor_reduce(out=gmax[:], in_=gboth[:],
                                    axis=mybir.AxisListType.X, op=mybir.AluOpType.max)

            # conv6: h6_c = LReLU(W6b_c @ xc + (W6a_c @ gmax + t6_c))
            h6 = [lrpool.tile([P128, HALF], F32, tag="hbuf", name="h6_0"),
                  sbp.tile([P128, HALF], F32, tag="h6_1", name="h6_1")]
            for c in range(2):
                vp = ps.tile([P128, 64], F32, tag="ps")
                nc.tensor.matmul(vp[:, 0:1], w6as[:, c * 128:(c + 1) * 128], gmax[:],
                                 start=True, stop=True)
                bias6 = sb.tile([P128, 1], F32, tag="bias6")
                nc.vector.tensor_add(out=bias6[:], in0=vp[:, 0:1], in1=t6s[:, c:c + 1])
                for j in range(HALF // 512):
                    pp = ps.tile([P128, 512], F32, tag="ps")
                    nc.tensor.matmul(pp[:], _r(w6bs[:, c * 128:(c + 1) * 128]),
                                     _r(xcown[:, j * 512:(j + 1) * 512]),
                                     start=True, stop=True)
                    sl6 = h6[c][:, j * 512:(j + 1) * 512]
                    nc.scalar.activation(sl6, pp[:], mybir.ActivationFunctionType.Identity,
                                         bias=bias6[:, 0:1])
                    _lrelu(nc, sl6, sl6)

            # conv9: out = W9 @ h6  (contraction 256 in 2 chunks)
            osb = stg.tile([1, HALF], F32, tag="osb")
            for j in range(HALF // 512):
                pp = ps.tile([1, 512], F32, tag="ps")
                for c in range(2):
                    nc.tensor.matmul(pp[:], w9s[:, c:c + 1],
                                     h6[c][:, j * 512:(j + 1) * 512],
                                     start=(c == 0), stop=(c == 1))
                nc.scalar.copy(osb[:, j * 512:(j + 1) * 512], pp[:])
            nc.sync.dma_start(out[:], osb[:])

    nc.finalize()
    return nc


def _fold_weights(i):
    f = np.float32
    o = {}
    s1 = (i["g1"] / np.sqrt(i["v1"] + EPS)).astype(f)
    A1 = (i["W1"][:, 0:2] * s1[:, None]).astype(f)
    B1 = ((i["W1"][:, 2:4] - i["W1"][:, 0:2]) * s1[:, None]).astype(f)
    t1 = (i["b1"] - i["m1"] * s1).astype(f)
    o["a1t"] = np.ascontiguousarray(A1.T)
    b1t = np.zeros((5, 64), f)
    b1t[0:2] = (0.5 / S1) * B1.T
    b1t[3] = t1 / S1
    o["b1t"] = b1t
    s2 = (i["g2"] / np.sqrt(i["v2"] + EPS)).astype(f)
    A2 = (i["W2"][:, 0:64] * s2[:, None]).astype(f)
    B2 = ((i["W2"][:, 64:128] - i["W2"][:, 0:64]) * s2[:, None]).astype(f)
    t2 = (i["b2"] - i["m2"] * s2).astype(f)
    o["a2t"] = np.ascontiguousarray(A2.T)
    b2t = np.zeros((66, 64), f)
    b2t[0:64] = B2.T
    b2t[65] = t2
    o["b2t"] = b2t
    s5 = (i["g5"] / np.sqrt(i["v5"] + EPS)).astype(f)
    W5s = (i["W5"] * s5[:, None]).astype(f)
    o["w5t"] = np.ascontiguousarray(W5s.T)
    o["t5"] = (i["b5"] - i["m5"] * s5).astype(f).reshape(128, 1)
    s6 = (i["g6"] / np.sqrt(i["v6"] + EPS)).astype(f)
    W6s = (i["W6"] * s6[:, None]).astype(f)
    o["w6at"] = np.ascontiguousarray(W6s[:, 0:128].T)
    o["w6bt"] = np.ascontiguousarray(W6s[:, 128:256].T)
    t6 = (i["b6"] - i["m6"] * s6).astype(f)
    o["t6"] = np.ascontiguousarray(t6.reshape(2, 128).T)
    o["w9t"] = np.ascontiguousarray(i["W9"].reshape(2, 128).T)
    return o


def kernel(**inputs):
    inputs = {k: np.asarray(v, np.float32) for k, v in inputs.items()}
    if "nc" not in _BUILT:
        _BUILT["nc"] = build()
    nc = _BUILT["nc"]
    w = _fold_weights(inputs)
    x = inputs["x"]
    in_maps = []
    for c in range(NCORES):
        b, h = c // 2, c % 2
        m = dict(w)
        m["xq"] = np.ascontiguousarray(x[b][:, h * HALF:(h + 1) * HALF])
        m["xc"] = np.ascontiguousarray(x[b])
        in_maps.append(m)
    res = run_bass_kernel_spmd(nc, in_maps, core_ids=list(range(NCORES)))
    out = np.zeros((B, N), np.float32)
    for c in range(NCORES):
        b, h = c // 2, c % 2
        out[b, h * HALF:(h + 1) * HALF] = res.results[c]["out"][0]
    return out


# revision 13
# speedup vs baseline: 1.0045x; 1.0045x over previous
"""DGCNN-simple Trainium2 kernel (v2: chunked quantized-encode KNN).

Strategy (8 NeuronCores, B=4 samples):
  core c -> sample b = c//2, query half h = c%2 (2048 queries each).
  Per EdgeConv, conv+BN+LReLU+max over K neighbors folded via LReLU
  monotonicity:
     x_out(o,n) = LReLU( max_k P(o, j_k(n)) + Q(o,n) )
  with P = (s*A) @ x (neighbor part, DRAM table) and Q = (s*(B-A)) @ x + t.

  KNN (the hot loop) uses an approximate-but-validated scheme:
  the pd matmul computes  S*pd + 1.5*2^32  (scale S and the big constant
  folded into the contraction as extra rows; the big constant is the LAST
  contraction row).  fp32 rounding in the [2^32, 2^33) binade quantizes the
  PSUM value to multiples of 512 "for free".  One scalar_tensor_tensor per
  512-chunk then computes  d = (c - (1.5*2^32 - 2^23)) + (j mod 512)
  which embeds the within-chunk column index into the low 9 bits exactly.
  A single max8 per 512-chunk (8 DVE passes of 512 = 1 full-width pass
  instead of 8) gives 64 candidates; a narrow 3-round top-20 merge over
  the 64 candidates recovers winners; local index = d mod 512 and chunk =
  merge_position//8 reconstruct the global neighbor index exactly.
  Validated vs the exact reference in numpy: rel err 1.25e-3 (gate 2e-2);
  per-row kNN set mismatch 0.22/0.24 of 20.

  Neighbor gather: ONE dma_gather (2560 indices) per query tile instead of
  20 indirect DMAs; fold max via a single strided tensor_reduce.
  Wide matmuls run as float32r (1 cycle/row vs 4 for fp32).
"""

import numpy as np
import concourse.bass as bass
import concourse.bacc as bacc
import concourse.mybir as mybir
import concourse.tile as tile
from concourse.bass_utils import run_bass_kernel_spmd
from concourse.masks import make_identity

N = 4096
K = 20
B = 4
EPS = 1e-5
SLOPE = 0.2
NCORES = 8
HALF = N // 2
P128 = 128
NT = HALF // P128  # query tiles per core
NCH = 8            # knn chunks per row
CW = N // NCH      # chunk width (512)
F32 = mybir.dt.float32
F32R = mybir.dt.float32r
I16 = mybir.dt.int16
U32 = mybir.dt.uint32

# knn quantization scales (validated in numpy): s1=1024, s2=64
S1 = 512.0 * 1024.0
S2 = 512.0 * 64.0
BIG = 1.5 * 2.0**32          # 6442450944, exactly representable
KSUB = BIG - 2.0**23         # subtracted on the encode op
NEG = -1e30

# engine split tuning: how many of the NCH chunk-encodes run on DVE (rest Pool)
ENC_DVE = 0

_BUILT = {}


def _lrelu(nc, out_ap, in_ap):
    nc.vector.scalar_tensor_tensor(out=out_ap, in0=in_ap, scalar=SLOPE, in1=in_ap,
                                   op0=mybir.AluOpType.mult, op1=mybir.AluOpType.max)


def _r(ap):
    return ap.bitcast(F32R)


def _edgeconv_phase(nc, tc, pools, *, lhs, rhs, cdim, ptab_dram, bq_rhs, bq_cdim,
                    lhsq, out_cb, iotaf, dram, ksubv, q_f32r):
    """One EdgeConv over NT query tiles."""
    sb, ps, pd_ps, stg, sbbig = pools
    _rq = _r if q_f32r else (lambda ap: ap)
    for t in range(NT):
        q0 = t * P128
        # chunk top-8 (encoded d values carry quantized pd + local idx).
        # gpsimd can't read PSUM: Act moves psum->SBUF with -KSUB folded in,
        # Pool adds the iota in place, DVE max8 scans each 512 chunk.
        V = sb.tile([P128, NCH * 8], F32, tag="V")
        for c4 in range(NCH // 2):
            c0 = c4 * 2 * CW
            pq = pd_ps.tile([P128, 2 * CW], F32, tag="pdq")
            for j in range(2):
                nc.tensor.matmul(pq[:, j * CW:(j + 1) * CW],
                                 _r(lhs[0:cdim, q0:q0 + P128]),
                                 _r(rhs[0:cdim, c0 + j * CW:c0 + (j + 1) * CW]),
                                 start=True, stop=True)
            dch = sbbig.tile([P128, 2 * CW], F32, tag="dch")
            nc.scalar.activation(dch[:], pq[:],
                                 mybir.ActivationFunctionType.Identity,
                                 bias=ksubv[:, 0:1])
            nc.gpsimd.tensor_tensor(out=dch[:], in0=dch[:], in1=iotaf[:],
                                    op=mybir.AluOpType.add)
            for j in range(2):
                nc.vector.max(out=V[:, (2 * c4 + j) * 8:(2 * c4 + j) * 8 + 8],
                              in_=dch[:, j * CW:(j + 1) * CW])
        # narrow 3-round top-24 merge with positions
        sel = sb.tile([P128, 24], F32, tag="sel")
        pos = sb.tile([P128, 24], U32, tag="pos")
        for r in range(3):
            nc.vector.max(out=sel[:, 8 * r:8 * r + 8], in_=V[:])
            nc.vector.max_index(out=pos[:, 8 * r:8 * r + 8],
                                in_max=sel[:, 8 * r:8 * r + 8], in_values=V[:])
            if r < 2:
                nc.vector.match_replace(out=V[:], in_to_replace=sel[:, 8 * r:8 * r + 8],
                                        in_values=V[:], imm_value=NEG)
        # reconstruct global indices with bit ops only (mod is not a valid
        # TensorScalar op): global = ((pos>>3)<<9) | (int(d) & 511)
        du = sb.tile([P128, K], U32, tag="du")
        nc.vector.tensor_scalar(out=du[:], in0=sel[:, 0:K], scalar1=0.0,
                                scalar2=None, op0=mybir.AluOpType.add)
        locu = sb.tile([P128, K], U32, tag="locu")
        nc.vector.tensor_scalar(out=locu[:], in0=du[:], scalar1=511,
                                scalar2=None, op0=mybir.AluOpType.bitwise_and)
        cbu = sb.tile([P128, K], U32, tag="cbu")
        nc.vector.tensor_scalar(out=cbu[:], in0=pos[:, 0:K], scalar1=3,
                                scalar2=9, op0=mybir.AluOpType.logical_shift_right,
                                op1=mybir.AluOpType.logical_shift_left)
        gi16 = sb.tile([P128, K], mybir.dt.uint16, tag="gi16")
        nc.vector.tensor_tensor(out=gi16[:], in0=cbu[:], in1=locu[:],
                                op=mybir.AluOpType.bitwise_or)
        # wrapped-idx layout via DRAM hop:
        # 1) shuffled write: idxd[(q%16)*160 + k*8 + q//16] = gi16[q][k]
        # 2) read back into (128,160): stride-0 leading dim replicates x8
        idxd = dram.tile([1, 16 * 8 * K], I16, tag="idxd")
        nc.sync.dma_start(
            idxd[:].rearrange("one (p k s0) -> one s0 p k", p=16, k=K, s0=8),
            gi16[:].bitcast(I16))
        idx_sb = sbbig.tile([P128, 8 * K], I16, tag="idxsb")
        nc.sync.dma_start(idx_sb[:],
                          idxd[:].broadcast_to([8, 16 * 8 * K]))
        # one gather for all 20 neighbors x 128 queries
        gath = sbbig.tile([P128, K * 64], F32, tag="gath")
        nc.gpsimd.dma_gather(out_ap=gath[:].rearrange("p (k o) -> p k o", k=K),
                             in_ap=ptab_dram[:], idxs_ap=idx_sb[:],
                             num_idxs=K * P128, num_idxs_reg=K * P128,
                             elem_size=64)
        # fold max over k (log-tree on Pool; DVE is the hot engine)
        def fmax(d0, s0, n):
            nc.gpsimd.tensor_tensor(out=gath[:, d0:d0 + n], in0=gath[:, d0:d0 + n],
                                    in1=gath[:, s0:s0 + n], op=mybir.AluOpType.max)
        fmax(0, 640, 640)    # k0-9  |= k10-19
        fmax(0, 320, 320)    # b0-4  |= b5-9
        fmax(0, 256, 64)     # b0    |= b4
        fmax(0, 128, 128)    # b0,b1 |= b2,b3
        fmax(0, 64, 64)      # b0    |= b1
        m = gath[:, 0:64]
        qp = ps.tile([P128, 64], F32, tag="ps")
        nc.tensor.matmul(qp[:], _rq(lhsq[0:bq_cdim, q0:q0 + P128]),
                         _rq(bq_rhs[0:bq_cdim, :]), start=True, stop=True)
        nc.vector.tensor_add(out=m[:], in0=m[:], in1=qp[:])
        xt = sb.tile([P128, 65], F32, tag="xoq", name="xoq")
        _lrelu(nc, xt[:, 0:64], m[:])
        out_cb(t, xt)


def _ptable(nc, pools, a_rhs, cdim, src, identity, ptab_dram, ptab_stage):
    """P = A^T-weights @ src -> transpose -> DRAM table (N, 64)."""
    sb, ps, pd_ps, stg, sbbig = pools
    p_sb = stg.tile([64, N], F32, tag="p_sb")
    for j in range(N // 512):
        pp = ps.tile([64, 512], F32, tag="ps")
        nc.tensor.matmul(pp[:], _r(a_rhs[0:cdim, :]), _r(src[0:cdim, j * 512:(j + 1) * 512]),
                         start=True, stop=True)
        nc.scalar.copy(p_sb[:, j * 512:(j + 1) * 512], pp[:])
    for j1 in range(N // P128):
        tp = ps.tile([P128, P128], F32, tag="ps")
        nc.tensor.transpose(tp[:, 0:64], p_sb[:, j1 * P128:(j1 + 1) * P128],
                            identity[0:64, 0:64])
        nc.scalar.copy(ptab_stage[:, j1 * 64:(j1 + 1) * 64], tp[:, 0:64])
    nc.sync.dma_start(
        ptab_dram[:].rearrange("(j1 j0) o -> j0 j1 o", j0=P128),
        ptab_stage[:].rearrange("p (j1 o) -> p j1 o", o=64))


def build():
    nc = bacc.Bacc(None, target_bir_lowering=False)
    dt = F32
    # ---- per-core inputs ----
    xq = nc.dram_tensor("xq", [2, HALF], dt, kind="ExternalInput")
    xc = nc.dram_tensor("xc", [2, N], dt, kind="ExternalInput")
    a1t = nc.dram_tensor("a1t", [2, 64], dt, kind="ExternalInput")
    b1t = nc.dram_tensor("b1t", [5, 64], dt, kind="ExternalInput")
    a2t = nc.dram_tensor("a2t", [64, 64], dt, kind="ExternalInput")
    b2t = nc.dram_tensor("b2t", [66, 64], dt, kind="ExternalInput")
    w5t = nc.dram_tensor("w5t", [128, 128], dt, kind="ExternalInput")
    t5 = nc.dram_tensor("t5", [128, 1], dt, kind="ExternalInput")
    w6at = nc.dram_tensor("w6at", [128, 256], dt, kind="ExternalInput")
    w6bt = nc.dram_tensor("w6bt", [128, 256], dt, kind="ExternalInput")
    t6 = nc.dram_tensor("t6", [128, 2], dt, kind="ExternalInput")
    w9t = nc.dram_tensor("w9t", [128, 2], dt, kind="ExternalInput")
    out = nc.dram_tensor("out", [1, HALF], dt, kind="ExternalOutput")

    with tile.TileContext(nc) as tc:
        with tc.tile_pool(name="sb", bufs=3) as sb, \
             tc.tile_pool(name="sbbig", bufs=3) as sbbig, \
             tc.tile_pool(name="lr", bufs=1) as lrpool, \
             tc.tile_pool(name="stg", bufs=1) as stg, \
             tc.tile_pool(name="sbp", bufs=1) as sbp, \
             tc.tile_pool(name="ps", bufs=2, space="PSUM") as ps, \
             tc.tile_pool(name="pdps", bufs=3, space="PSUM") as pd_ps, \
             tc.tile_pool(name="dram", bufs=3, space="DRAM") as dram:
            pools = (sb, ps, pd_ps, stg, sbbig)
            ident = sbp.tile([P128, P128], F32, tag="ident")
            make_identity(nc, ident[:])

            ksubv = sbp.tile([P128, 1], F32, tag="ksubv")
            nc.vector.memset(ksubv[:], -KSUB)
            # iota constant: (j mod 512) over a 1024 window, fp32 exact
            iotaf = sbp.tile([P128, 2 * CW], F32, tag="iotaf")
            nc.gpsimd.iota(iotaf[:].rearrange("p (a b) -> p a b", b=CW),
                           pattern=[[0, 2], [1, CW]], base=0,
                           channel_multiplier=0,
                           allow_small_or_imprecise_dtypes=True)

            # ---------- load inputs / small weights ----------
            def load(pool, dr, shape, tag, rr=False):
                t = pool.tile(list(shape), F32, tag=tag)
                if rr:
                    nc.sync.dma_start(_r(t[:]), _r(dr[:]))
                else:
                    nc.sync.dma_start(t[:], dr[:])
                return t

            a1s = load(sbp, a1t, (2, 64), "a1", rr=True)
            b1s = load(sbp, b1t, (5, 64), "b1", rr=True)
            a2s = load(sbp, a2t, (64, 64), "a2", rr=True)
            b2s = load(sbp, b2t, (66, 64), "b2")
            w5s = load(sbp, w5t, (128, 128), "w5", rr=True)
            t5s = load(sbp, t5, (128, 1), "t5")
            w6as = load(sbp, w6at, (128, 256), "w6a")
            w6bs = load(sbp, w6bt, (128, 256), "w6b", rr=True)
            t6s = load(sbp, t6, (128, 2), "t6")
            w9s = load(sbp, w9t, (128, 2), "w9")

            # ---------- phase A prep ----------
            # lhsA (5, HALF): [2*S1*xq(2); -S1*xxq; S1; BIG]
            # rhsA (5, N):    [xc(2); ones; -xxc; ones]
            lhsA = lrpool.tile([67, HALF], F32, tag="lhsbuf")
            rhsA = lrpool.tile([67, N], F32, tag="rhsbuf")
            nc.sync.dma_start(_r(lhsA[0:2, :]), _r(xq[:]))
            nc.sync.dma_start(_r(rhsA[0:2, :]), _r(xc[:]))
            # const rows: engine ops can't start at odd partitions -> build on
            # partition 0 and DMA into place
            ones_row = sbp.tile([1, N], F32, tag="ones_row")
            nc.vector.memset(ones_row[:], 1.0)
            crow = stg.tile([1, HALF], F32, tag="crow", name="crowS1")
            nc.vector.memset(crow[:], S1)
            nc.sync.dma_start(_r(rhsA[2:3, :]), _r(ones_row[:]))
            nc.sync.dma_start(_r(rhsA[4:5, :]), _r(ones_row[:]))
            nc.sync.dma_start(_r(lhsA[3:4, :]), _r(crow[:]))
            crow2 = stg.tile([1, HALF], F32, tag="crow", name="crowBIG")
            nc.vector.memset(crow2[:], BIG)
            nc.sync.dma_start(_r(lhsA[4:5, :]), _r(crow2[:]))
            # squared norms via ones-matmul on PE, chunked
            ones2 = sbp.tile([2, 1], F32, tag="ones2")
            nc.vector.memset(ones2[:], 1.0)
            for src_ap, dst_row, ncols, scl, nmname in (
                    (lhsA, lhsA[2:3, :], HALF, -1.0 / (4.0 * S1), "nrmq"),
                    (rhsA, rhsA[3:4, :], N, -1.0, "nrmc")):
                nrow = stg.tile([1, ncols], F32, tag="nrow", name=nmname)
                for j in range(ncols // 512):
                    sl = slice(j * 512, (j + 1) * 512)
                    sqt = stg.tile([2, 512], F32, tag="sqt", name="sqt")
                    nc.vector.tensor_mul(sqt[:], src_ap[0:2, sl], src_ap[0:2, sl])
                    pp = ps.tile([1, 512], F32, tag="ps", name="pp")
                    nc.tensor.matmul(pp[:], ones2[:], sqt[:], start=True, stop=True)
                    nc.scalar.activation(nrow[:, sl], pp[:],
                                         mybir.ActivationFunctionType.Copy, scale=scl)
                nc.sync.dma_start(_r(dst_row[:]), _r(nrow[:]))
            # query rows arrive pre-scaled by 2*S1 from the host

            # ---------- P1 table ----------
            p1d = dram.tile([N, 64], F32, tag="p1d")
            pstage = stg.tile([P128, (N // P128) * 64], F32, tag="pstage")
            _ptable(nc, pools, a1s, 2, rhsA, ident, p1d, pstage)

            # ---------- EdgeConv 1 ----------
            x1aug = sbp.tile([66, HALF], F32, tag="x1aug")

            def cb1(t, xt):
                sq = sb.tile([P128, 64], F32, tag="sq64", name="sq64")
                nc.vector.tensor_mul(sq[:], xt[:, 0:64], xt[:, 0:64])
                nc.vector.tensor_reduce(out=xt[:, 64:65], in_=sq[:],
                                        axis=mybir.AxisListType.X,
                                        op=mybir.AluOpType.add, negate=True)
                tp = ps.tile([P128, P128], F32, tag="ps", name="tp1")
                nc.tensor.transpose(tp[0:65, :], xt[:, 0:65], ident[:])
                nc.scalar.copy(x1aug[0:65, t * P128:(t + 1) * P128], tp[0:65, :])

            _edgeconv_phase(nc, tc, pools, lhs=lhsA, rhs=rhsA, cdim=5,
                            ptab_dram=p1d, bq_rhs=b1s, bq_cdim=5, lhsq=lhsA,
                            out_cb=cb1, iotaf=iotaf, dram=dram, ksubv=ksubv,
                            q_f32r=True)
            nc.sync.dma_start(x1aug[65:66, :], ones_row[:, 0:HALF])

            # ---------- exchange halves (AllGather over core pairs) ----------
            ccin = dram.tile([66, HALF], F32, tag="ccin")
            ccout = dram.tile([132, HALF], F32, tag="ccout")
            nc.sync.dma_start(ccin[:], x1aug[0:66, :])
            nc.gpsimd.collective_compute(
                "AllGather", mybir.AluOpType.bypass,
                replica_groups=[[0, 1], [2, 3], [4, 5], [6, 7]],
                ins=[ccin.opt()], outs=[ccout.opt()])

            # ---------- phase B prep ----------
            # rhsB (67, N): [x1_full(64); ones; -xx1_full; ones]
            rhsB = lrpool.tile([67, N], F32, tag="rhsbuf")
            nc.sync.dma_start(_r(rhsB[0:64, 0:HALF]), _r(ccout[0:64, :]))
            nc.sync.dma_start(_r(rhsB[0:64, HALF:N]), _r(ccout[66:130, :]))
            nc.sync.dma_start(_r(rhsB[65:66, 0:HALF]), _r(ccout[64:65, :]))
            nc.sync.dma_start(_r(rhsB[65:66, HALF:N]), _r(ccout[130:131, :]))
            nc.sync.dma_start(_r(rhsB[64:65, :]), _r(ones_row[:]))
            nc.sync.dma_start(_r(rhsB[66:67, :]), _r(ones_row[:]))
            # lhsB (67, HALF): [2*S2*x1_own(64); -S2*xx2_own; S2; BIG]
            # built fp32 in staging; SBUF->SBUF DMA is the f32r producer
            lhsBs = stg.tile([67, HALF], F32, tag="lhsBs")
            nc.vector.tensor_scalar_mul(lhsBs[0:64, :], x1aug[0:64, :], 2.0 * S2)
            nc.vector.tensor_scalar_mul(lhsBs[64:65, :], x1aug[64:65, :], S2)
            crow3 = stg.tile([1, HALF], F32, tag="crow", name="crowS2")
            nc.vector.memset(crow3[:], S2)
            nc.sync.dma_start(lhsBs[65:66, :], crow3[:])
            crow4 = stg.tile([1, HALF], F32, tag="crow", name="crowBIG2")
            nc.vector.memset(crow4[:], BIG)
            nc.sync.dma_start(lhsBs[66:67, :], crow4[:])
            lhsB = lrpool.tile([67, HALF], F32, tag="lhsbuf")
            nc.sync.dma_start(_r(lhsB[:]), _r(lhsBs[:]))

            # ---------- P2 table ----------
            p2d = dram.tile([N, 64], F32, tag="p2d")
            _ptable(nc, pools, a2s, 64, rhsB, ident, p2d, pstage)

            # ---------- EdgeConv 2 ----------
            xcown = sbp.tile([P128, HALF], F32, tag="xcown")
            nc.sync.dma_start(_r(xcown[0:64, :]), _r(x1aug[0:64, :]))

            def cb2(t, xt):
                tp = ps.tile([P128, P128], F32, tag="ps", name="tp2")
                nc.tensor.transpose(tp[0:64, :], xt[:, 0:64], ident[:])
                x2s = sb.tile([64, P128], F32, tag="x2s", name="x2s")
                nc.scalar.copy(x2s[:], tp[0:64, :])
                nc.sync.dma_start(_r(xcown[64:128, t * P128:(t + 1) * P128]), _r(x2s[:]))

            _edgeconv_phase(nc, tc, pools, lhs=lhsB, rhs=rhsB, cdim=67,
                            ptab_dram=p2d, bq_rhs=b2s, bq_cdim=66, lhsq=x1aug,
                            out_cb=cb2, iotaf=iotaf, dram=dram, ksubv=ksubv,
                            q_f32r=False)

            # ---------- tail ----------
            # conv5 + global max (partial over own half)
            h5 = lrpool.tile([P128, HALF], F32, tag="hbuf")
            for j in range(HALF // 512):
                pp = ps.tile([P128, 512], F32, tag="ps")
                nc.tensor.matmul(pp[:], _r(w5s[:]), _r(xcown[:, j * 512:(j + 1) * 512]),
                                 start=True, stop=True)
                sl5 = h5[:, j * 512:(j + 1) * 512]
                nc.scalar.activation(sl5, pp[:], mybir.ActivationFunctionType.Identity,
                                     bias=t5s[:, 0:1])
                _lrelu(nc, sl5, sl5)
            gpart = sb.tile([P128, 1], F32, tag="gpart")
            nc.vector.tensor_reduce(out=gpart[:], in_=h5[:],
                                    axis=mybir.AxisListType.X, op=mybir.AluOpType.max)
            gin = dram.tile([P128, 1], F32, tag="gin")
            gout = dram.tile([256, 1], F32, tag="gout")
            nc.sync.dma_start(gin[:], gpart[:])
            nc.gpsimd.collective_compute(
                "AllGather", mybir.AluOpType.bypass,
                replica_groups=[[0, 1], [2, 3], [4, 5], [6, 7]],
                ins=[gin.opt()], outs=[gout.opt()])
            gboth = sb.tile([P128, 2], F32, tag="gboth")
            nc.sync.dma_start(gboth[:, 0:1], gout[0:P128, :])
            nc.sync.dma_start(gboth[:, 1:2], gout[P128:256, :])
            gmax = sb.tile([P128, 1], F32, tag="gmax")
            nc.vector.tensor_reduce(out=gmax[:], in_=gboth[:],
                                    axis=mybir.AxisListType.X, op=mybir.AluOpType.max)

            # conv6: h6_c = LReLU(W6b_c @ xc + (W6a_c @ gmax + t6_c))
            h6 = [lrpool.tile([P128, HALF], F32, tag="hbuf", name="h6_0"),
                  sbp.tile([P128, HALF], F32, tag="h6_1", name="h6_1")]
            for c in range(2):
                vp = ps.tile([P128, 64], F32, tag="ps")
                nc.tensor.matmul(vp[:, 0:1], w6as[:, c * 128:(c + 1) * 128], gmax[:],
                                 start=True, stop=True)
                bias6 = sb.tile([P128, 1], F32, tag="bias6")
                nc.vector.tensor_add(out=bias6[:], in0=vp[:, 0:1], in1=t6s[:, c:c + 1])
                for j in range(HALF // 512):
                    pp = ps.tile([P128, 512], F32, tag="ps")
                    nc.tensor.matmul(pp[:], _r(w6bs[:, c * 128:(c + 1) * 128]),
                                     _r(xcown[:, j * 512:(j + 1) * 512]),
                                     start=True, stop=True)
                    sl6 = h6[c][:, j * 512:(j + 1) * 512]
                    nc.scalar.activation(sl6, pp[:], mybir.ActivationFunctionType.Identity,
                                         bias=bias6[:, 0:1])
                    _lrelu(nc, sl6, sl6)

            # conv9: out = W9 @ h6  (contraction 256 in 2 chunks)
            osb = stg.tile([1, HALF], F32, tag="osb")
            for j in range(HALF // 512):
                pp = ps.tile([1, 512], F32, tag="ps")
                for c in range(2):
                    nc.tensor.matmul(pp[:], w9s[:, c:c + 1],
                                     h6[c][:, j * 512:(j + 1) * 512],
                                     start=(c == 0), stop=(c == 1))
                nc.scalar.copy(osb[:, j * 512:(j + 1) * 512], pp[:])
            nc.sync.dma_start(out[:], osb[:])

    nc.finalize()
    return nc


def _fold_weights(i):
    f = np.float32
    o = {}
    s1 = (i["g1"] / np.sqrt(i["v1"] + EPS)).astype(f)
    A1 = (i["W1"][:, 0:2] * s1[:, None]).astype(f)
    B1 = ((i["W1"][:, 2:4] - i["W1"][:, 0:2]) * s1[:, None]).astype(f)
    t1 = (i["b1"] - i["m1"] * s1).astype(f)
    o["a1t"] = np.ascontiguousarray(A1.T)
    b1t = np.zeros((5, 64), f)
    b1t[0:2] = (0.5 / S1) * B1.T
    b1t[3] = t1 / S1
    o["b1t"] = b1t
    s2 = (i["g2"] / np.sqrt(i["v2"] + EPS)).astype(f)
    A2 = (i["W2"][:, 0:64] * s2[:, None]).astype(f)
    B2 = ((i["W2"][:, 64:128] - i["W2"][:, 0:64]) * s2[:, None]).astype(f)
    t2 = (i["b2"] - i["m2"] * s2).astype(f)
    o["a2t"] = np.ascontiguousarray(A2.T)
    b2t = np.zeros((66, 64), f)
    b2t[0:64] = B2.T
    b2t[65] = t2
    o["b2t"] = b2t
    s5 = (i["g5"] / np.sqrt(i["v5"] + EPS)).astype(f)
    W5s = (i["W5"] * s5[:, None]).astype(f)
    o["w5t"] = np.ascontiguousarray(W5s.T)
    o["t5"] = (i["b5"] - i["m5"] * s5).astype(f).reshape(128, 1)
    s6 = (i["g6"] / np.sqrt(i["v6"] + EPS)).astype(f)
    W6s = (i["W6"] * s6[:, None]).astype(f)
    o["w6at"] = np.ascontiguousarray(W6s[:, 0:128].T)
    o["w6bt"] = np.ascontiguousarray(W6s[:, 128:256].T)
    t6 = (i["b6"] - i["m6"] * s6).astype(f)
    o["t6"] = np.ascontiguousarray(t6.reshape(2, 128).T)
    o["w9t"] = np.ascontiguousarray(i["W9"].reshape(2, 128).T)
    return o


def kernel(**inputs):
    inputs = {k: np.asarray(v, np.float32) for k, v in inputs.items()}
    if "nc" not in _BUILT:
        _BUILT["nc"] = build()
    nc = _BUILT["nc"]
    w = _fold_weights(inputs)
    x = inputs["x"]
    in_maps = []
    for c in range(NCORES):
        b, h = c // 2, c % 2
        m = dict(w)
        m["xq"] = np.ascontiguousarray(2.0 * S1 * x[b][:, h * HALF:(h + 1) * HALF])
        m["xc"] = np.ascontiguousarray(x[b])
        in_maps.append(m)
    res = run_bass_kernel_spmd(nc, in_maps, core_ids=list(range(NCORES)))
    out = np.zeros((B, N), np.float32)
    for c in range(NCORES):
        b, h = c // 2, c % 2
        out[b, h * HALF:(h + 1) * HALF] = res.results[c]["out"][0]
    return out
